# revision 20
# baseline (speedup 1.0000x reference)
"""GQA attention layer (dense_transformer) on 8 Trainium2 NeuronCores.

Sharding: data-parallel over batch (2) x tensor-parallel over head groups (4).
Core c handles batch c//4 and head-group c%4 (8 q heads, 2 kv heads).
Each core computes a partial output (its heads' contribution through its
Wo row-slice); the host sums the 4 partials per batch.

Per-core pipeline (all matmuls bf16, fp32 accumulation):
  Warmup: dummy matmuls on the identity tile while weights stream in, so the
       PE HAM clock-gate is already at 8/8 when real work starts. A dummy
       exp preloads the ACT exp table so P2's first exp isn't gated on it.
  P1 (fused QKV): one pass over the 16 token tiles; per tile the kt-loop
       accumulates Q (2x512) and KV (512) psums; RMSNorm (rstd applied after
       the rope muls), RoPE via per-tile streamed cos/sin tables, transpose
       per head -> qT/kT [d, i]; V copied token-major into v_aug with a ones
       column appended for softmax denominators. For it=15 the psums are
       first evicted to SBUF by fast ACT copies so the psum banks free
       ~1.4us after the last P1 matmul (avoids a PE idle gap at the phase
       boundary that used to re-throttle the HAM clock gate to 4/8).
  P2+P3 (merged, software-pipelined): per (ib, h): scoresT[j,i] psum in
       [128,1024] chunks (causal trimming on diagonal tiles), exp on
       ScalarE, 128-wide diag masks on VectorE; the PV matmuls + normalize
       + transpose of head h-1 AND output-projection (Wo) matmul groups for
       completed token blocks are interleaved between head h's score chunks
       so the PE never stalls on the Scalar-engine exp. Wo groups evict via
       GpSimd/Vector and DMA out per (m, quarter). The Wo matmul stream is
       the elastic filler that keeps the PE at streaming roofline through
       the exp-bound attention phase; leftover Wo groups drain densely at
       the end.
"""
import math
import os
import sys
from contextlib import ExitStack

import numpy as np

_REPO = "/opt/trn_rl_repo"
_PKGS = "/opt/pypackages"
for _p in (_REPO, _PKGS):
    if _p not in sys.path:
        sys.path.append(_p)

import ml_dtypes

BF16 = ml_dtypes.bfloat16

B, S, HIDDEN = 2, 2048, 4096
NUM_HEADS, NUM_KV_HEADS, HEAD_DIM = 32, 8, 128
EPS = 1e-6
ROPE_THETA = 10000.0
N_CORES = 8
TP = 4  # head groups
HQ = NUM_HEADS // TP        # 8 q heads per core
HKV = NUM_KV_HEADS // TP    # 2 kv heads per core
KT = HIDDEN // 128          # 32 k tiles
IT = S // 128               # 16 token tiles
IB = S // 512               # 4 token blocks (512 wide)


def _split_drain_waits():
    """walrus here rejects >1 sync wait on the tail Drain; split them."""
    from concourse import mybir
    from concourse.tile import TileContext
    from concourse.vector_clock import ScopedClock

    def _drain_and_barrier(self, tick_clock, wait_clock):
        drain_inst = self.nc.sync.drain()
        wait_clock.add_sem_waits(
            drain_inst.ins, ScopedClock({None: tick_clock.global_clock})
        )
        inst = drain_inst.ins
        si = inst.sync_info
        if si is not None and si.on_wait is not None and len(si.on_wait) > 1:
            waits = list(si.on_wait)
            del si.on_wait[1:]
            for i in range(1, len(waits)):
                e_inst = self.nc.sync.drain().ins
                if e_inst.sync_info is None:
                    e_inst.sync_info = mybir.SyncInfo(on_wait=[], on_update=[])
                e_inst.sync_info.on_wait.extend(waits[i : i + 1])
        self.nc.all_engine_barrier()
        assert self.sems is not None
        popped = self.nc._tile_sem_poison_stack.pop()
        assert popped is self._sem_poison
        self.nc.clear_and_free_semaphores(list(self.sems.allocated().values()))
        self.nc.all_engine_barrier()
        _fixup_wait_limits(self.nc)

    TileContext._drain_and_barrier = _drain_and_barrier


def _fixup_wait_limits(nc):
    """walrus in this image caps sync waits per instruction (DMA: hit at 3,
    Drain at 4+). Hoist excess waits onto nop instructions inserted just
    before the offender on the same engine (waits still complete before the
    original program point; engine order preserves semantics)."""
    from concourse import mybir

    def limit_for(inst):
        return 1

    def mk_nop(engine):
        bi = nc.engines[engine].nop(nofuse=True)
        inst = bi.ins if hasattr(bi, "ins") else bi
        for f in nc.m.functions:
            for blk in f.blocks:
                if blk.instructions and blk.instructions[-1] is inst:
                    blk.instructions.pop()
        return inst

    for f in nc.m.functions:
        for blk in f.blocks:
            out = []
            for inst in blk.instructions:
                si = inst.sync_info
                nw = len(si.on_wait) if si is not None and si.on_wait else 0
                lim = limit_for(inst)
                if nw > lim:
                    waits = list(si.on_wait)
                    del si.on_wait[lim:]
                    for w in waits[lim:]:
                        nop = mk_nop(inst.engine)
                        nop.sync_info = mybir.SyncInfo(on_wait=[w], on_update=[])
                        out.append(nop)
                out.append(inst)
            blk.instructions[:] = out


def build_bass():
    import concourse.bass as bass
    import concourse.tile as tile
    from concourse import mybir

    _split_drain_waits()

    f32 = mybir.dt.float32
    bf16 = mybir.dt.bfloat16
    AF = mybir.ActivationFunctionType
    ALU = mybir.AluOpType

    nc = bass.Bass("TRN2", target_bir_lowering=False, debug=False)

    hst = nc.dram_tensor("hst", [IT, 128, KT, 128], bf16, kind="ExternalInput")
    wq = nc.dram_tensor("wq", [128, KT, HQ * 128], bf16, kind="ExternalInput")
    wkv = nc.dram_tensor("wkv", [128, KT, 4 * 128], bf16, kind="ExternalInput")
    wo = nc.dram_tensor("wo", [128, HQ, HIDDEN], bf16, kind="ExternalInput")
    # packed per-token-tile rope tables: cols [cosq|sinq|cosk|sink]
    costab = nc.dram_tensor("costab", [128, IT, 512], bf16, kind="ExternalInput")
    masks = nc.dram_tensor("masks", [128, 4, 512], bf16, kind="ExternalInput")
    out = nc.dram_tensor("out", [S, HIDDEN], f32, kind="ExternalOutput")

    with tile.TileContext(nc) as tc, ExitStack() as top:
        const = top.enter_context(tc.tile_pool(name="const", bufs=1))
        res = top.enter_context(tc.tile_pool(name="res", bufs=1))

        # identity built on-chip (iota + compare) so the PE warmup isn't
        # gated on the first DMA (~8us queue spin-up)
        ident_sb = const.tile([128, 128], bf16, tag="ident")
        iota_r = const.tile([128, 128], mybir.dt.int16, tag="iota_r")
        iota_c = const.tile([128, 128], mybir.dt.int16, tag="iota_c")
        nc.gpsimd.iota(iota_r, pattern=[[0, 128]], channel_multiplier=1)
        nc.gpsimd.iota(iota_c, pattern=[[1, 128]], channel_multiplier=0)
        nc.vector.tensor_tensor(out=ident_sb, in0=iota_r, in1=iota_c,
                                op=ALU.is_equal)
        eps_sb = const.tile([128, 1], f32, tag="eps")
        nc.vector.memset(eps_sb, EPS)
        # preload the ACT table (ln/exp/square/copy all live in the
        # natural_log_exp_and_others set; rstd uses exp(-0.5*ln(x)) instead
        # of sqrt so the whole kernel needs a single table -> no reloads)
        expwarm = const.tile([128, 1], f32, tag="expwarm")
        nc.scalar.activation(out=expwarm, in_=eps_sb, func=AF.Exp)

        # causal diag masks live for the whole kernel; DMA'd at start
        masks_sb = res.tile([128, 4, 512], bf16, tag="masks")

        qkv_stack = ExitStack()
        qkv_res = qkv_stack.enter_context(tc.tile_pool(name="qkv_res", bufs=1))
        qT = qkv_res.tile([128, HQ, S], bf16, tag="qT")
        kT = qkv_res.tile([128, HKV, S], bf16, tag="kT")
        v_aug = qkv_res.tile([128, HKV, IT, 129], bf16, tag="vaug")
        # stage for it=15's psums (fast psum eviction at the P1->P2
        # boundary); the whole it=15 rope tail runs as P2 filler off this
        # stage so every P1 pool zone releases right at the last matmul.
        pstage = qkv_res.tile([128, 1536], bf16, tag="pstage")
        rqtail = qkv_res.tile([128, 1280], bf16, tag="rqtail")
        cs15_sb = qkv_res.tile([128, 512], bf16, tag="cs15")
        tail_tiles = []
        for gi, nh in ((0, 4), (1, 4), (2, 2)):
            tail_tiles.append({
                k: qkv_res.tile([128, 4, 64], bf16, tag=f"{k}15_{gi}",
                                name=f"{k}15_{gi}")
                for k in ("ta", "tb", "th", "t2")
            })
            tail_tiles[gi]["sq"] = qkv_res.tile(
                [128, 512], bf16, tag=f"sq15_{gi}", name=f"sq15_{gi}"
            )
            tail_tiles[gi]["ss"] = qkv_res.tile(
                [128, 4], f32, tag=f"ss15_{gi}", name=f"ss15_{gi}"
            )
            tail_tiles[gi]["rstd"] = qkv_res.tile(
                [128, 4], f32, tag=f"rstd15_{gi}", name=f"rstd15_{gi}"
            )
        nc.vector.memset(v_aug[:, :, :, 128:129], 1.0)

        # ---------------- Phase 1: fused QKV projection ----------------
        with ExitStack() as p1:
            wqpool = p1.enter_context(tc.tile_pool(name="wq", bufs=1))
            wkvpool = p1.enter_context(tc.tile_pool(name="wkv", bufs=1))
            hpool = p1.enter_context(tc.tile_pool(name="hst", bufs=2))
            cspool = p1.enter_context(tc.tile_pool(name="cs", bufs=2))
            qpsum = p1.enter_context(tc.tile_pool(name="qpsum", bufs=2, space="PSUM"))
            trans_psum = p1.enter_context(
                tc.tile_pool(name="tpsum", bufs=2, space="PSUM")
            )
            stage_a = p1.enter_context(tc.tile_pool(name="stage_a", bufs=3))
            stage = p1.enter_context(tc.tile_pool(name="stage", bufs=1))
            small = p1.enter_context(tc.tile_pool(name="small", bufs=4))

            # ht tiles (bufs=2 rotation); DMAs for it=0,1 emitted before the
            # weight chunks so compute can begin ASAP.
            ht_tiles = [
                hpool.tile([128, KT, 128], bf16, tag="ht", name=f"ht{it}")
                for it in range(IT)
            ]
            wq_sb = wqpool.tile([128, KT, HQ * 128], bf16, tag="wq")
            wkv_sb = wkvpool.tile([128, KT, 512], bf16, tag="wkv")
            cs_tiles = [
                cspool.tile([128, 512], bf16, tag="cs", name=f"cs{it}")
                for it in range(IT - 1)
            ]
            # fine-grained interleave of the first chunk (per-kt slices) so
            # the kt stream of it=0 can start after ~0.5MB instead of 2MB
            for kt in range(4):
                nc.sync.dma_start(
                    out=ht_tiles[0][:, kt * 8 : (kt + 1) * 8, :],
                    in_=hst.ap()[0, :, kt * 8 : (kt + 1) * 8, :],
                )
                nc.sync.dma_start(
                    out=wq_sb[:, kt : kt + 1, :], in_=wq.ap()[:, kt : kt + 1, :]
                )
                nc.sync.dma_start(
                    out=wkv_sb[:, kt : kt + 1, :],
                    in_=wkv.ap()[:, kt : kt + 1, :],
                )
            # rope tables for the first two tiles come early: the it=0 rope
            # chain must not be gated on the full weight stream (psum
            # rotation for it=2 waits on it=0's psum readers)
            nc.sync.dma_start(out=cs_tiles[0], in_=costab.ap()[:, 0, :])
            nc.sync.dma_start(out=cs_tiles[1], in_=costab.ap()[:, 1, :])
            # kt-ordered weight chunks (4 kt each): matmul kt stream unblocks
            # progressively.
            for c in range(1, 8):
                nc.sync.dma_start(
                    out=wq_sb[:, c * 4 : (c + 1) * 4, :],
                    in_=wq.ap()[:, c * 4 : (c + 1) * 4, :],
                )
                nc.sync.dma_start(
                    out=wkv_sb[:, c * 4 : (c + 1) * 4, :],
                    in_=wkv.ap()[:, c * 4 : (c + 1) * 4, :],
                )
                if c == 1:
                    nc.sync.dma_start(out=ht_tiles[1], in_=hst.ap()[1])
            nc.sync.dma_start(out=masks_sb, in_=masks.ap())
            nc.sync.dma_start(out=cs15_sb, in_=costab.ap()[:, IT - 1, :])

            # PE warmup: dummy matmuls on ident while weights stream in.
            for w in range(32):
                warm = qpsum.tile([128, 512], f32, tag="psq0", name=f"warm{w}")
                nc.tensor.matmul(
                    warm[:, 0:128], ident_sb, ident_sb, start=True, stop=True
                )

            def rope_phase_a(psum_t, n_heads, cs_t, cs_off, it, h_base):
                """All psum-READING ops for one head group: 4 rope muls (DVE)
                + square (ACT). Emitted for every group of an iteration
                BEFORE any dependent chain so the psum banks free as early
                as possible. Returns a context for rope_phase_b."""
                w = n_heads * 128
                x3 = psum_t.rearrange("p (h d) -> p h d", h=n_heads)
                cos_t = cs_t[:, cs_off : cs_off + 128]
                sin_t = cs_t[:, cs_off + 128 : cs_off + 256]
                ct = cos_t[:, 0:64][:, None, :].broadcast_to([128, n_heads, 64])
                cb = cos_t[:, 64:128][:, None, :].broadcast_to([128, n_heads, 64])
                st_ = sin_t[:, 0:64][:, None, :].broadcast_to([128, n_heads, 64])
                sb_ = sin_t[:, 64:128][:, None, :].broadcast_to([128, n_heads, 64])
                ta = stage_a.tile(
                    [128, 4, 64], bf16, tag="ta", name=f"ta_{it}_{h_base}_{cs_off}"
                )
                tb = stage_a.tile(
                    [128, 4, 64], bf16, tag="tb", name=f"tb_{it}_{h_base}_{cs_off}"
                )
                th = stage_a.tile(
                    [128, 4, 64], bf16, tag="th", name=f"th_{it}_{h_base}_{cs_off}"
                )
                t2 = stage_a.tile(
                    [128, 4, 64], bf16, tag="t2", name=f"t2_{it}_{h_base}_{cs_off}"
                )
                sq = stage_a.tile(
                    [128, 512], bf16, tag="sq", name=f"sq_{it}_{h_base}_{cs_off}"
                )
                nc.vector.tensor_mul(out=ta[:, 0:n_heads], in0=x3[:, :, 0:64], in1=ct)
                nc.vector.tensor_mul(out=tb[:, 0:n_heads], in0=x3[:, :, 64:128], in1=st_)
                nc.vector.tensor_mul(out=th[:, 0:n_heads], in0=x3[:, :, 64:128], in1=cb)
                nc.vector.tensor_mul(out=t2[:, 0:n_heads], in0=x3[:, :, 0:64], in1=sb_)
                nc.scalar.activation(out=sq[:, 0:w], in_=psum_t, func=AF.Square)
                return (n_heads, w, it, h_base, cs_off, ta, tb, th, t2, sq)

            def rope_phase_b(ctx, dst, rq_tile=None):
                """Dependent chain: combine rope halves, rstd, scale, and
                transpose each head to dst [d, i]. If rq_tile is given the
                transposes are deferred and (src, dst) pairs returned."""
                n_heads, w, it, h_base, cs_off, ta, tb, th, t2, sq = ctx
                nc.vector.tensor_sub(
                    out=ta[:, 0:n_heads], in0=ta[:, 0:n_heads], in1=tb[:, 0:n_heads]
                )
                nc.vector.tensor_add(
                    out=tb[:, 0:n_heads], in0=th[:, 0:n_heads], in1=t2[:, 0:n_heads]
                )
                ss = small.tile([128, 4], f32, tag="ss", name=f"ss_{it}_{h_base}_{cs_off}")
                nc.vector.tensor_reduce(
                    out=ss[:, 0:n_heads],
                    in_=sq[:, 0:w].rearrange("p (h d) -> p h d", h=n_heads),
                    op=ALU.add, axis=mybir.AxisListType.X,
                )
                rstd = small.tile(
                    [128, 4], f32, tag="rstd", name=f"rstd_{it}_{h_base}_{cs_off}"
                )
                nc.scalar.activation(
                    out=rstd[:, 0:n_heads], in_=ss[:, 0:n_heads], func=AF.Ln,
                    scale=1.0 / HEAD_DIM, bias=eps_sb,
                )
                nc.scalar.activation(
                    out=rstd[:, 0:n_heads], in_=rstd[:, 0:n_heads], func=AF.Exp,
                    scale=-0.5,
                )
                if rq_tile is None:
                    rq = stage.tile(
                        [128, 512], bf16, tag="rq", name=f"rq_{it}_{h_base}_{cs_off}"
                    )
                else:
                    rq = rq_tile
                rq3 = rq[:, 0:w].rearrange("p (h d) -> p h d", h=n_heads)
                for h in range(n_heads):
                    nc.vector.tensor_scalar_mul(
                        out=rq3[:, h, 0:64], in0=ta[:, h, :],
                        scalar1=rstd[:, h : h + 1],
                    )
                    nc.vector.tensor_scalar_mul(
                        out=rq3[:, h, 64:128], in0=tb[:, h, :],
                        scalar1=rstd[:, h : h + 1],
                    )
                pairs = [
                    (
                        rq[:, h * 128 : (h + 1) * 128],
                        dst[:, h_base + h, it * 128 : (it + 1) * 128],
                    )
                    for h in range(n_heads)
                ]
                if rq_tile is not None:
                    return pairs
                for src, dsl in pairs:
                    ps_t = trans_psum.tile(
                        [128, 128], bf16,
                        tag="ps_t", name=f"ps_t_{it}_{h_base}_{cs_off}",
                    )
                    nc.tensor.transpose(ps_t, src, ident_sb)
                    nc.vector.tensor_copy(out=dsl, in_=ps_t)
                return None

            for it in range(IT):
                if it >= 2:
                    nc.sync.dma_start(out=ht_tiles[it], in_=hst.ap()[it])
                if 1 <= it + 1 < IT - 1 and it >= 1:
                    nc.sync.dma_start(
                        out=cs_tiles[it + 1], in_=costab.ap()[:, it + 1, :]
                    )
                ht = ht_tiles[it]
                ps = [
                    qpsum.tile([128, 512], f32, tag=f"psq{j}", name=f"psq{j}_{it}")
                    for j in range(2)
                ]
                pkv = qpsum.tile([128, 512], f32, tag="pskv", name=f"pskv_{it}")
                for kt in range(KT):
                    st = kt == 0
                    sp = kt == KT - 1
                    nc.tensor.matmul(ps[0][:], ht[:, kt, :], wq_sb[:, kt, 0:512],
                                     start=st, stop=sp)
                    nc.tensor.matmul(ps[1][:], ht[:, kt, :], wq_sb[:, kt, 512:1024],
                                     start=st, stop=sp)
                    nc.tensor.matmul(pkv[:], ht[:, kt, :], wkv_sb[:, kt, :],
                                     start=st, stop=sp)
                if it == IT - 1:
                    # evict the psums to SBUF with fast ACT copies so the
                    # psum banks (which the P2 score psums overlap) free
                    # immediately; the whole rope tail + v_aug copy for this
                    # tile is deferred into P2's interleave (tail units).
                    nc.scalar.activation(out=pstage[:, 0:512], in_=ps[0],
                                         func=AF.Copy)
                    nc.vector.tensor_copy(out=pstage[:, 512:1024], in_=ps[1])
                    nc.vector.tensor_copy(out=pstage[:, 1024:1536], in_=pkv)
                    continue
                for g in range(HKV):
                    sl = pkv[:, 256 + g * 128 : 256 + g * 128 + 128]
                    nc.scalar.activation(
                        out=v_aug[:, g, it, 0:128], in_=sl, func=AF.Copy
                    )
                # phase A for all three groups first: psum banks free early
                ctx0 = rope_phase_a(ps[0][:, :], 4, cs_tiles[it], 0, it, 0)
                ctx1 = rope_phase_a(ps[1][:, :], 4, cs_tiles[it], 0, it, 4)
                ctx2 = rope_phase_a(pkv[:, 0:256], 2, cs_tiles[it], 256, it, 0)
                rope_phase_b(ctx0, qT)
                rope_phase_b(ctx1, qT)
                rope_phase_b(ctx2, kT)

        # -------- Phase 2+3 merged: attention + output projection --------
        # Pool creation order controls which released-P1 SBUF zones each
        # pool overlaps: wo/aoT/pts land on the early-released weight/hst
        # zones (readers done at the last P1 matmul); ostage/stage2 land on
        # the late-released stage_a zone (rope-tail readers, ~+4us) but are
        # first written well after that.
        late = ExitStack()
        wopool = late.enter_context(tc.tile_pool(name="wo", bufs=1))
        aopool = late.enter_context(tc.tile_pool(name="aores", bufs=1))
        wo_sb = wopool.tile([128, HQ, HIDDEN], bf16, tag="wo")
        aoT = aopool.tile([128, HQ, S], bf16, tag="aoT")
        # quarter-major chunks: the first Wo-projection groups (quarter 0)
        # become runnable as soon as the first chunk lands.
        for q in range(4):
            nc.sync.dma_start(
                out=wo_sb[:, :, q * 1024 : (q + 1) * 1024],
                in_=wo.ap()[:, :, q * 1024 : (q + 1) * 1024],
            )

        with ExitStack() as p2:
            # PSUM budget (8 banks, bank-granular per tile):
            #   spsum  3 x [128, 512] f32  -> banks 0-2 (score chunks)
            #   opsum  2 x [128, 256] f32  -> banks 3-4 (po uses [:,0:129])
            #   tpsum2 1 x [128, 128] bf16 -> bank  5
            #   p3um   2 x [128, 512] f32  -> banks 6-7
            spsum = p2.enter_context(tc.tile_pool(name="spsum", bufs=3, space="PSUM"))
            opsum = p2.enter_context(tc.tile_pool(name="opsum", bufs=2, space="PSUM"))
            tpsum2 = p2.enter_context(
                tc.tile_pool(name="tpsum2", bufs=1, space="PSUM")
            )
            p3um = p2.enter_context(tc.tile_pool(name="p3um", bufs=1, space="PSUM"))
            ptpool = p2.enter_context(tc.tile_pool(name="pt", bufs=1))
            stage2 = p2.enter_context(tc.tile_pool(name="stage2", bufs=3))
            small2 = p2.enter_context(tc.tile_pool(name="small2", bufs=4))
            ostage = p2.enter_context(tc.tile_pool(name="ostage", bufs=2))

            pts_tiles = [
                ptpool.tile([128, IT * 512], bf16, tag=f"pts{par}", name=f"pts{par}")
                for par in range(2)
            ]

            def chunk_layout(ib):
                """Packed pts/psum layout for block ib: key tile jt stores
                its trimmed range [lo(jt), 512) of the i-block at running
                column offset C(jt). Trimming keeps the exp element count
                minimal on the Scalar engine."""
                lay = []
                c = 0
                for jt in range(4 * ib + 4):
                    r = jt - 4 * ib
                    lo = 128 * r if r > 0 else 0
                    lay.append((jt, r, lo, c, 512 - lo))
                    c += 512 - lo
                return lay

            def emit_scores(ib, h, lay):
                """Score chunks + exp + diag masks for (ib, h); one psum
                tile + exp per chunk. Returns emit-points; caller
                interleaves filler units."""
                g = h // (HQ // HKV)
                pts = pts_tiles[h % 2]
                slots = []
                for (jt, r, lo, c, wd) in lay:

                    def s_unit(jt=jt, r=r, lo=lo, c=c, wd=wd, g=g, h=h,
                               ib=ib, pts=pts):
                        ps_s = spsum.tile(
                            [128, 512], f32, tag="ps_s",
                            name=f"ps_s_{ib}_{h}_{jt}",
                        )
                        nc.tensor.matmul(
                            ps_s[:, 0:wd],
                            kT[:, g, jt * 128 : (jt + 1) * 128],
                            qT[:, h, ib * 512 + lo : (ib + 1) * 512],
                            start=True,
                            stop=True,
                        )
                        nc.scalar.activation(
                            out=pts[:, c : c + wd],
                            in_=ps_s[:, 0:wd],
                            func=AF.Exp,
                        )
                        # only the first 128 stored cols of a diag chunk can
                        # contain masked positions (beyond that jj<=ii-128r
                        # holds for every jj)
                        if r >= 0:
                            nc.vector.tensor_mul(
                                out=pts[:, c : c + 128],
                                in0=pts[:, c : c + 128],
                                in1=masks_sb[:, r, lo : lo + 128],
                            )
                    slots.append(s_unit)
                return slots

            def pv_units(ib, h, lay):
                """PV + normalize + transpose for (ib, h) as a flat list of
                small closures (interleaved between next head's scores)."""
                g = h // (HQ // HKV)
                pts = pts_tiles[h % 2]
                units = []
                for itl in range(4):
                    it_g = ib * 4 + itl
                    po_box = {}

                    def mk_mm(jt, itl=itl, it_g=it_g, po_box=po_box, g=g, h=h,
                              ib=ib, pts=pts, lay=lay):
                        def f():
                            if jt == 0:
                                po_box["po"] = opsum.tile(
                                    [128, 256], f32, tag="po",
                                    name=f"po_{ib}_{h}_{itl}",
                                )
                            _, r, lo, c, wd = lay[jt]
                            col = c + itl * 128 - lo
                            nc.tensor.matmul(
                                po_box["po"][:, 0:129],
                                pts[:, col : col + 128],
                                v_aug[:, g, jt, :],
                                start=(jt == 0),
                                stop=(jt == it_g),
                            )
                        return f

                    for jt in range(it_g + 1):
                        units.append(mk_mm(jt))

                    def fin(itl=itl, it_g=it_g, po_box=po_box, h=h, ib=ib):
                        po = po_box["po"]
                        rec = small2.tile(
                            [128, 1], f32, tag="rec", name=f"rec_{h}_{it_g}"
                        )
                        nc.vector.reciprocal(out=rec, in_=po[:, 128:129])
                        ao = stage2.tile(
                            [128, 128], bf16, tag="ao", name=f"ao_{h}_{it_g}"
                        )
                        nc.vector.tensor_scalar_mul(
                            out=ao, in0=po[:, 0:128], scalar1=rec
                        )
                        ps_t = tpsum2.tile(
                            [128, 128], bf16, tag="ps_t2", name=f"ps_t2_{h}_{it_g}"
                        )
                        nc.tensor.transpose(ps_t, ao, ident_sb)
                        nc.vector.tensor_copy(
                            out=aoT[:, h, it_g * 128 : (it_g + 1) * 128],
                            in_=ps_t,
                        )
                    units.append(fin)
                return units

            def p3_units_for_block(b):
                """Output-projection groups for token block b (m tiles
                4b..4b+3). Each unit: 8 accumulating N=512 matmuls (one per
                head) + eviction; the pair of units per (m, quarter) shares
                an ostage tile and DMAs out together. Quarter-major order so
                early groups only need the first wo DMA chunk."""
                units = []
                for q in range(4):
                    for mi in range(4):
                        m = b * 4 + mi
                        box = {}

                        def u0(m=m, q=q, box=box):
                            box["st"] = ostage.tile(
                                [128, 1024], f32, tag="ost", name=f"ost_{m}_{q}"
                            )
                            po = p3um.tile(
                                [128, 512], f32, tag="po3a", name=f"po3a_{m}_{q}"
                            )
                            for k in range(HQ):
                                nc.tensor.matmul(
                                    po[:],
                                    aoT[:, k, m * 128 : (m + 1) * 128],
                                    wo_sb[:, k, q * 1024 : q * 1024 + 512],
                                    start=(k == 0),
                                    stop=(k == HQ - 1),
                                )
                            nc.scalar.activation(
                                out=box["st"][:, 0:512], in_=po, func=AF.Copy
                            )

                        def u1(m=m, q=q, box=box):
                            po = p3um.tile(
                                [128, 512], f32, tag="po3b", name=f"po3b_{m}_{q}"
                            )
                            for k in range(HQ):
                                nc.tensor.matmul(
                                    po[:],
                                    aoT[:, k, m * 128 : (m + 1) * 128],
                                    wo_sb[:, k, q * 1024 + 512 : (q + 1) * 1024],
                                    start=(k == 0),
                                    stop=(k == HQ - 1),
                                )
                            nc.vector.tensor_copy(
                                out=box["st"][:, 512:1024], in_=po
                            )
                            nc.sync.dma_start(
                                out=out.ap()[
                                    m * 128 : (m + 1) * 128,
                                    q * 1024 : (q + 1) * 1024,
                                ],
                                in_=box["st"],
                            )
                        units += [u0, u1]
                return units

            # it=15's full rope tail (norm + rope + v_aug + transposes) runs
            # as P2 interleave filler off the pstage copy. Emitted during
            # (1,*) where the Wo-projection filler keeps the PE fed; its
            # results are first needed by (3,*) scores/PV. The square runs
            # on DVE and the three sqrts share one closure so the ScalarE
            # exp table is swapped out at most once.
            def make_tail_units():
                it = IT - 1
                specs = [
                    (0, pstage[:, 0:512], 4, 0, 0, qT, rqtail[:, 0:512]),
                    (1, pstage[:, 512:1024], 4, 0, 4, qT,
                     rqtail[:, 512:1024]),
                    (2, pstage[:, 1024:1280], 2, 256, 0, kT,
                     rqtail[:, 1024:1280]),
                ]
                units = []

                def mk_a(gi, src, nh, cs_off):
                    def a():
                        t = tail_tiles[gi]
                        w = nh * 128
                        x3 = src.rearrange("p (h d) -> p h d", h=nh)
                        cos_t = cs15_sb[:, cs_off : cs_off + 128]
                        sin_t = cs15_sb[:, cs_off + 128 : cs_off + 256]
                        ct = cos_t[:, 0:64][:, None, :].broadcast_to([128, nh, 64])
                        cb = cos_t[:, 64:128][:, None, :].broadcast_to([128, nh, 64])
                        st_ = sin_t[:, 0:64][:, None, :].broadcast_to([128, nh, 64])
                        sb_ = sin_t[:, 64:128][:, None, :].broadcast_to([128, nh, 64])
                        nc.vector.tensor_mul(out=t["ta"][:, 0:nh],
                                             in0=x3[:, :, 0:64], in1=ct)
                        nc.vector.tensor_mul(out=t["tb"][:, 0:nh],
                                             in0=x3[:, :, 64:128], in1=st_)
                        nc.vector.tensor_mul(out=t["th"][:, 0:nh],
                                             in0=x3[:, :, 64:128], in1=cb)
                        nc.vector.tensor_mul(out=t["t2"][:, 0:nh],
                                             in0=x3[:, :, 0:64], in1=sb_)
                        nc.vector.tensor_mul(out=t["sq"][:, 0:w],
                                             in0=src, in1=src)
                    return a

                def mk_b1(gi, nh):
                    def b1():
                        t = tail_tiles[gi]
                        w = nh * 128
                        nc.vector.tensor_sub(out=t["ta"][:, 0:nh],
                                             in0=t["ta"][:, 0:nh],
                                             in1=t["tb"][:, 0:nh])
                        nc.vector.tensor_add(out=t["tb"][:, 0:nh],
                                             in0=t["th"][:, 0:nh],
                                             in1=t["t2"][:, 0:nh])
                        nc.vector.tensor_reduce(
                            out=t["ss"][:, 0:nh],
                            in_=t["sq"][:, 0:w].rearrange(
                                "p (h d) -> p h d", h=nh),
                            op=ALU.add, axis=mybir.AxisListType.X,
                        )
                    return b1

                def sqrts():
                    for gi, _, nh, _, _, _, _ in specs:
                        t = tail_tiles[gi]
                        nc.scalar.activation(
                            out=t["rstd"][:, 0:nh], in_=t["ss"][:, 0:nh],
                            func=AF.Ln, scale=1.0 / HEAD_DIM, bias=eps_sb,
                        )
                    for gi, _, nh, _, _, _, _ in specs:
                        t = tail_tiles[gi]
                        nc.scalar.activation(
                            out=t["rstd"][:, 0:nh], in_=t["rstd"][:, 0:nh],
                            func=AF.Exp, scale=-0.5,
                        )

                def mk_mul(gi, nh, rq):
                    def mul():
                        t = tail_tiles[gi]
                        rq3 = rq.rearrange("p (h d) -> p h d", h=nh)
                        for h in range(nh):
                            nc.vector.tensor_scalar_mul(
                                out=rq3[:, h, 0:64], in0=t["ta"][:, h, :],
                                scalar1=t["rstd"][:, h : h + 1],
                            )
                            nc.vector.tensor_scalar_mul(
                                out=rq3[:, h, 64:128], in0=t["tb"][:, h, :],
                                scalar1=t["rstd"][:, h : h + 1],
                            )
                    return mul

                def vcopy():
                    for g in range(HKV):
                        sl = pstage[:, 1280 + g * 128 : 1280 + (g + 1) * 128]
                        nc.scalar.activation(
                            out=v_aug[:, g, it, 0:128], in_=sl, func=AF.Copy
                        )

                def mk_tp(src, dsl, k):
                    def f():
                        ps_t = tpsum2.tile(
                            [128, 128], bf16, tag="ps_t2", name=f"ps_tail_{k}"
                        )
                        nc.tensor.transpose(ps_t, src, ident_sb)
                        nc.vector.tensor_copy(out=dsl, in_=ps_t)
                    return f

                for (gi, src, nh, cs_off, _, _, _) in specs:
                    units.append(mk_a(gi, src, nh, cs_off))
                units.append(vcopy)
                for (gi, _, nh, _, _, _, _) in specs:
                    units.append(mk_b1(gi, nh))
                units.append(sqrts)
                k = 0
                for (gi, _, nh, _, h_base, dst, rq) in specs:
                    units.append(mk_mul(gi, nh, rq))
                    for h in range(nh):
                        units.append(mk_tp(
                            rq[:, h * 128 : (h + 1) * 128],
                            dst[:, h_base + h, it * 128 : (it + 1) * 128],
                            k,
                        ))
                        k += 1
                return units

            tail_units = make_tail_units()
            pending = []
            p3q = []
            for ib in range(IB):
                lay = chunk_layout(ib)
                for h in range(HQ):
                    if h == 1 and ib >= 1:
                        # block ib-1's aoT tiles are complete (its last pv
                        # fins were flushed during (ib, h=0))
                        p3q.extend(p3_units_for_block(ib - 1))
                    quota = 0 if ib == 0 else (3 + ib)
                    take, p3q = p3q[:quota], p3q[quota:]
                    fillers = pending + take
                    slots = emit_scores(ib, h, lay)
                    nsl = len(slots)
                    for i, s in enumerate(slots):
                        s()
                        if fillers:
                            hi = -(-len(fillers) * (i + 1) // nsl)  # ceil split
                            lo_i = -(-len(fillers) * i // nsl)
                            for u in fillers[lo_i:hi]:
                                u()
                    pending = pv_units(ib, h, lay)
                    if ib == 1 and h == 0:
                        pending = pending + tail_units
            for u in pending:
                u()
            p3q.extend(p3_units_for_block(IB - 1))
            for u in p3q:
                u()
        late.close()
        qkv_stack.close()

    return nc


def prep_core_inputs(hidden_states, position_ids, Wq, Wk, Wv, Wo, q_norm_w, k_norm_w):
    """Host-side shard + layout prep. Returns list of 8 in_maps."""
    pos = np.asarray(position_ids).reshape(-1).astype(np.float64)  # [S]
    inv_freq = 1.0 / (
        ROPE_THETA ** (np.arange(0, HEAD_DIM, 2, dtype=np.float64) / HEAD_DIM)
    )  # [64]
    ang = pos[:, None] * inv_freq[None, :]  # [S, 64]
    emb = np.concatenate([ang, ang], axis=1)  # [S, 128]
    scale = HEAD_DIM ** (-0.25)
    cos = (np.cos(emb) * scale).astype(np.float32)  # [S, 128]
    sin = (np.sin(emb) * scale).astype(np.float32)
    qw = np.asarray(q_norm_w, dtype=np.float32)
    kw = np.asarray(k_norm_w, dtype=np.float32)
    qw_roll = np.concatenate([qw[64:], qw[:64]])
    kw_roll = np.concatenate([kw[64:], kw[:64]])

    def table(t):  # [S,128] -> [128, IT, 128]
        return np.ascontiguousarray(
            t.reshape(IT, 128, 128).transpose(1, 0, 2)
        )

    cosq_t = table(cos * qw[None, :]).astype(BF16)
    sinq_t = table(sin * qw_roll[None, :]).astype(BF16)
    cosk_t = table(cos * kw[None, :]).astype(BF16)
    sink_t = table(sin * kw_roll[None, :]).astype(BF16)
    # pack per-it: [128, IT, 512] = [cosq | sinq | cosk | sink]
    costab = np.ascontiguousarray(
        np.concatenate([cosq_t, sinq_t, cosk_t, sink_t], axis=2)
    )

    # causal masks for the 4 diagonal offsets
    jj = np.arange(128)[:, None]
    ii = np.arange(512)[None, :]
    masks = np.stack(
        [(jj <= ii - 128 * r).astype(np.float32) for r in range(4)]
    ).transpose(1, 0, 2)  # [128, 4, 512]
    masks = masks.astype(BF16)

    hs = np.asarray(hidden_states, dtype=np.float32)
    Wq = np.asarray(Wq, dtype=np.float32)
    Wk = np.asarray(Wk, dtype=np.float32)
    Wv = np.asarray(Wv, dtype=np.float32)
    Wo = np.asarray(Wo, dtype=np.float32)

    hst_b = []
    for b in range(B):
        hsT = hs[b].T.astype(BF16)  # [4096, 2048]
        # -> [IT, 128(i), KT, 128(k)]: hst[it, ip, kt, kp] = hsT[kt*128+kp, it*128+ip]
        t = hsT.reshape(KT, 128, IT, 128).transpose(2, 1, 0, 3)
        hst_b.append(np.ascontiguousarray(t))

    in_maps = []
    for c in range(N_CORES):
        b, grp = divmod(c, TP)
        wq_s = Wq[:, grp * HQ * 128 : (grp + 1) * HQ * 128].astype(BF16)
        wq_t = np.ascontiguousarray(
            wq_s.reshape(KT, 128, HQ * 128).transpose(1, 0, 2)
        )  # [128, KT, 1024]
        wk_s = Wk[:, grp * HKV * 128 : (grp + 1) * HKV * 128]
        wv_s = Wv[:, grp * HKV * 128 : (grp + 1) * HKV * 128]
        wkv_s = np.concatenate([wk_s, wv_s], axis=1).astype(BF16)  # [4096, 512]
        wkv_t = np.ascontiguousarray(
            wkv_s.reshape(KT, 128, 512).transpose(1, 0, 2)
        )  # [128, KT, 512]
        wo_s = Wo[grp * HQ * 128 : (grp + 1) * HQ * 128, :].astype(BF16)  # [1024, 4096]
        wo_t = np.ascontiguousarray(
            wo_s.reshape(HQ, 128, HIDDEN).transpose(1, 0, 2)
        )  # [128, HQ, 4096]
        in_maps.append(
            {
                "hst": hst_b[b],
                "wq": wq_t,
                "wkv": wkv_t,
                "wo": wo_t,
                "costab": costab,
                "masks": masks,
            }
        )
    return in_maps


def kernel(hidden_states, position_ids, Wq, Wk, Wv, Wo, q_norm_w, k_norm_w,
           _trace=False, _tmpdir=None):
    from concourse.bass_utils import run_bass_kernel_spmd

    nc = build_bass()
    in_maps = prep_core_inputs(
        hidden_states, position_ids, Wq, Wk, Wv, Wo, q_norm_w, k_norm_w
    )
    kwargs = {}
    if _trace:
        kwargs = dict(trace=True, tmpdir=_tmpdir)
    res = run_bass_kernel_spmd(nc, in_maps, list(range(N_CORES)), **kwargs)
    partials = [res.results[c]["out"] for c in range(N_CORES)]
    outb = [
        np.sum([partials[b * TP + g] for g in range(TP)], axis=0, dtype=np.float32)
        for b in range(B)
    ]
    full = np.stack(outb).astype(np.float32)  # [2, 2048, 4096]
    if _trace:
        kernel._last_result = res
    return full
